# revision 4
# baseline (speedup 1.0000x reference)
"""Background-noise layer kernel for 8 Trainium2 NeuronCores.

Math (matches the reference): Poisson background spikes S (600, 10) with a
fixed RNG key, COO edge lists scattered into a dense weight matrix
W (250000, 10) (duplicates sum), output = S @ W^T reshaped to (1, 600, 250000).

Sharding: the neuron (output-feature) axis is split into 8 contiguous shards
of 31250. Each core holds its W-shard transposed (10, 31250) plus the tiny
replicated spike matrix transposed (10, 600), computes its (600, 31250) output
slice with TensorE matmuls (K=10 contraction on the partition axis), and
streams it straight out to DRAM. The kernel is output-write bound (~75 MB per
core), which is the memory roofline for this problem.

Host-side preprocessing is limited to RNG (the spikes are a constant — the
reference uses a fixed key 42, only implemented for threefry) and the COO →
dense scatter of the 1M edge weights (index bookkeeping; the dense W is what
the device GEMM consumes).
"""

import base64
import zlib

import numpy as np

B, T, U = 1, 600, 10
N_V1, N_LM = 200_000, 50_000
N_TOTAL = N_V1 + N_LM  # 250_000
N_CORES = 8
N_SHARD = N_TOTAL // N_CORES  # 31_250

# jax.random.poisson(jax.random.key(42, impl='threefry2x32'), 1.0, (600, 10))
# computed once offline; values are tiny ints (0..6). zlib+b64 of uint8 bytes.
_SPIKES_B64 = (
    "eJxNWAuy3DAIA+Ht/Y9co4/z+ma6u0nsYBCS7O7iv7mfP3TX/WwUevbH/X7/w73Ys3fQ9+peOntxr4N/"
    "++X02U89f5+rO8P9cp+ae2nHFSfcccUf99rh6zxrczq+hrPsYzeOnY0RQKGWn21Nowj2RQ1HN7PvvteG"
    "kdzQZtoD7wBsPHf4tEK/d/e1U6VkcGgf1NnRuxC9kolQNjbGgkPmrfndj8MrpfA2XmX0xgUmQdFjOJZL"
    "3JD3KWy6FFR3csTZD39gn9zYdyYtnhWpoyi4/vrm489WDqe1Ik4PZX4X384Il3yYCzg9SqvD3Tzea8P3"
    "Q0mCSsk5lJfO85mTd0eZI3yAMwpNEymFApLWqyQkOdgK8D4zAU89AuIXfDlaVWsD5Q0MRpPPvfGbnyJ3"
    "imdjcvjEwaD0HuOiIPwMZ6oKLHcUp95h+8/5dgbaw9QZ7ZaBcrQv/REeW0gmhi1wFIZhNa9T7gD4MScb"
    "Wr0wRNynHZiucjl0A6m9WnzfMgJ8YK5O3QinPcQV0/LrdSUbYdgreVK3TBLl3u0Uj9W6fQdM2kSdVA9M"
    "7Ag1N+rr+05fFHlI9WJU0/x1yi/aJzn7jGnLebpZVXCtDmyvlGSz0Dtqg1H2tor/RoXakkCrxHFXtRlS"
    "IbRTPHt3xF9KQqrh7GoR3UYuzA5oNwq4kBkTogu9te1HdOa4ffMpZ1HFfuQDM0XWzqgqqDH6hLDjJwhV"
    "hn5nI5ZZ2334iLuF2GHjhXIhGuoJuZIz+bYxXLc4wkSQT1TeBB8x+2jgj7MpvKcaYnoXLtzS5vJ2nkPC"
    "hJcGo93aqEd1kyf7VUc0KkL8qRZs53vxt9QEp9W1XOwKIOz1Pr+etJY4hksQB8AEgCdwgDvSjLeXz6ys"
    "aBliB0i+RtkYaYXR+spffokInpSjJFlh+3dcU4dFzbwg3t4DwhBILUdJkdRsyMffpgS9KwcE9uPx5WUx"
    "BURyXPN0Om5BE/oLEXwamRYqt5jbTDbjEIYprmFj8yHVMMaNjnq05CRE5yC3MJGegMsGQbSieRFS3PRP"
    "G/kdnBHHgtgIRSr79quW/VrLKDWFfWSsBSHsYjnH4I2rKLGCP7BqZT6pX+Z8+ooUzZJhUETL+o+IvWaQ"
    "GbMstrra7UkxK4qSSM+6r5gtxSQJ88n8WbOZztTw8W8ULkkW25C5xrzdf5ZisbRkuPyNBx8xmkON/wvD"
    "2nLQ2NCddNmSQgy14j3qYAuKvyW+XIL/qEtxkly6chyOadsrRwGRoCqM1KpNgmKdG8m/uFHZsDCNWMIv"
    "OCRU6oo8IVf93FVoroKrHpOI4DAauV6Oci4iwktYxZOm+SwwFfoWub9SWKS8cqiQWru91msZBc5MKAsi"
    "W/PKvliKXDYKUB8/ApL3rXEbxRhWf33UsqFPUVfEB/W3v5vlokSLw1zWeRwc48gBRMBK3dhA5QOPPZjm"
    "EdhHYr/v+bkvaEiqIl6ukIG07DhPe6CaZ4FsS/veL+GdWds23AkYfRfORm7Eb4vbmPKObbT/EqpQHw06"
    "DQcxpC4pm7JkO+wjVJXztZtNrfNJqTfdUDUgmxBgaF5vqVgOQFCY6IuZarrfdnDSd3LOvRKERzG89St"
    "ZjQhThwTLRSo9eKf6ab/obevk0XREsAqlpj+FS/c+U0E2JcG23CbsrrVtdEOct4v1bgWup7NqiuFeAnF"
    "sLNyE4b0uSqv3QPF+yrx2R8btyQ6DqmdggZW3sMREpzvwZOeUG7bg3mdFRCDDdhUzCSQTrTTWtHvekaM"
    "cupO5/5VKzDyCaff6wGG6bUjvP5uoO9OJdIwpyz7HtRNAype0dm7MjvWmrGumDdQTVKIsEAi1IxyGGNY"
    "y6WpwrEoH+uU9H0fMUY7lm6U4v5CGAM1V/HYdv/pbiKhwoIN4JZ80RMK2AbZfd2JDdepUiluVwxNmAPZ"
    "9E741yxAfM1FMfPCtnIF0LP0TrFuWEtnS3Ec/8I8zt3wfOyDRqqOe77bgx7SRBv7YEfDcIs3CUCd1AKl"
    "jEMj/6cnoAA1PI0cHnNG0bpfK13+eXIcJ2nDafpAhtYUzvhTIyS9vwyVmbs/Nr0hR8+vlzs9Tf0V8xBT"
    "4iv3tu2WA/HI3O0wHsUEBA56sbEinPygt5x0V7Bm1ehi398Lj2x4fw+lafU7AFNje082TXB6cMNUqHrX"
    "yeZDxDtpCNJ6PnagtSHv35e10xV25ExXfrK/e957VVdsdP/ng47OOcmVNSsLeOPdTNqCSwHhCm4t7/zz"
    "E5dhOz3JhUepiEBj4YM9d1abuboTbpQfauHgTE4yr1oOv9DFxIxueTpf2rgWvemdSWaoPOirWQNLxa+9"
    "jrJ1htM4BUuyjTg366Yrts5vEbjmSbhXtWBixaceDucqsOl3mCcNZNG3/6iBm7WVCh2netCJnU8oEbUL"
    "rmOH3eL0R4TUafG7Y3irK1MUQ5XBZ4x62be7+mKQ/53QbxdHHOH3a4+CjaUnRxNXMqWdWYoHiZnJsyCK"
    "JFXd1I6z001n8B+MpF8o="
)


def _spikes_t() -> np.ndarray:
    """Transposed spike matrix (U, T) float32."""
    raw = zlib.decompress(base64.b64decode(_SPIKES_B64))
    s = np.frombuffer(raw, dtype=np.uint8).astype(np.float32).reshape(T, U)
    return np.ascontiguousarray(s.T)


def _split_multi_waits(nc):
    """This environment's walrus rejects instructions carrying more than one
    sync-wait command ("Too many sync wait commands" in setupSyncWait). Tile
    freely attaches several waits to one instruction (e.g. a matmul waiting on
    two DMA-queue sems, or the kernel-tail drain waiting on every DMA lane).
    Post-pass: for every instruction with >1 wait, keep the first and move the
    rest onto fresh wait-only EventSemaphore instructions inserted immediately
    before it on the same engine. Waits are pre-execution conditions, so
    hoisting them onto same-engine predecessors inserted at that exact point
    preserves semantics."""
    import bass_rust

    ctr = 0
    for f in nc.m.functions:
        for bb in f.blocks:
            insts = bb.instructions  # live list
            new_list = None
            for ins in insts:
                si = getattr(ins, "sync_info", None)
                waits = list(si.on_wait) if si is not None else []
                if len(waits) > 1:
                    if new_list is None:
                        # copy of everything before this instruction
                        pos = insts.index(ins)
                        new_list = list(insts[:pos])
                    si.on_wait = [waits[0]]
                    for w in waits[1:]:
                        ctr += 1
                        ev = bass_rust.InstEventSemaphore(
                            name=f"wsplit_{ctr}",
                            engine=ins.engine,
                            ins=[],
                            outs=[],
                            sync_info=bass_rust.SyncInfo(on_wait=[w], on_update=[]),
                        )
                        new_list.append(ev)
                    new_list.append(ins)
                elif new_list is not None:
                    new_list.append(ins)
            if new_list is not None:
                insts[:] = new_list
    return ctr


_NC_CACHE = None


def build_nc():
    """Build the per-core Bass program: out(600, 31250) = spkT.T @ wT."""
    global _NC_CACHE
    if _NC_CACHE is not None:
        return _NC_CACHE

    import concourse.bass as bass
    import concourse.mybir as mybir
    from concourse.tile import TileContext

    f32 = mybir.dt.float32
    nc = bass.Bass(trn_type="TRN2")
    spk = nc.dram_tensor("spk", [U, T], f32, kind="ExternalInput")
    wt = nc.dram_tensor("wt", [U, N_SHARD], f32, kind="ExternalInput")
    out = nc.dram_tensor("out", [T, N_SHARD], f32, kind="ExternalOutput")

    NCHUNK = 512
    m_tiles = [(m0, min(128, T - m0)) for m0 in range(0, T, 128)]
    n_chunks = [(n0, min(NCHUNK, N_SHARD - n0)) for n0 in range(0, N_SHARD, NCHUNK)]

    with TileContext(nc) as tc:
        with (
            tc.tile_pool(name="const", bufs=1) as cpool,
            tc.tile_pool(name="stage", bufs=6) as stage,
            tc.tile_pool(name="psum", bufs=8, space="PSUM") as pp,
        ):
            spk_t = cpool.tile([U, T], f32)
            nc.sync.dma_start(out=spk_t[:], in_=spk[:])
            w_t = cpool.tile([U, N_SHARD], f32)
            nc.sync.dma_start(out=w_t[:], in_=wt[:])

            for m0, msz in m_tiles:
                for n0, nsz in n_chunks:
                    ps = pp.tile([128, NCHUNK], f32)
                    nc.tensor.matmul(
                        ps[:msz, :nsz],
                        lhsT=spk_t[:, m0 : m0 + msz],
                        rhs=w_t[:, n0 : n0 + nsz],
                        start=True,
                        stop=True,
                    )
                    ot = stage.tile([128, NCHUNK], f32)
                    nc.any.tensor_copy(out=ot[:msz, :nsz], in_=ps[:msz, :nsz])
                    nc.sync.dma_start(
                        out=out[m0 : m0 + msz, n0 : n0 + nsz], in_=ot[:msz, :nsz]
                    )

    _split_multi_waits(nc)
    _NC_CACHE = nc
    return nc


def make_in_maps(w_v1, rows_v1, cols_v1, w_lm, rows_lm, cols_lm):
    """Host preprocessing: scatter COO edges into dense W, shard, transpose."""
    w_v1 = np.asarray(w_v1, dtype=np.float32)
    w_lm = np.asarray(w_lm, dtype=np.float32)
    rows_v1 = np.asarray(rows_v1)
    cols_v1 = np.asarray(cols_v1)
    rows_lm = np.asarray(rows_lm)
    cols_lm = np.asarray(cols_lm)

    flat_v1 = rows_v1.astype(np.int64) * U + cols_v1.astype(np.int64)
    flat_lm = (rows_lm.astype(np.int64) + N_V1) * U + cols_lm.astype(np.int64)
    acc = np.bincount(flat_v1, weights=w_v1.astype(np.float64), minlength=N_TOTAL * U)
    acc += np.bincount(flat_lm, weights=w_lm.astype(np.float64), minlength=N_TOTAL * U)
    W = acc.astype(np.float32).reshape(N_TOTAL, U)

    spk_t = _spikes_t()
    in_maps = []
    for c in range(N_CORES):
        w_shard_t = np.ascontiguousarray(W[c * N_SHARD : (c + 1) * N_SHARD].T)
        in_maps.append({"spk": spk_t, "wt": w_shard_t})
    return in_maps


def kernel(inp, w_v1, rows_v1, cols_v1, w_lm, rows_lm, cols_lm):
    from concourse.bass_utils import run_bass_kernel_spmd

    nc = build_nc()
    in_maps = make_in_maps(w_v1, rows_v1, cols_v1, w_lm, rows_lm, cols_lm)
    res = run_bass_kernel_spmd(nc, in_maps, core_ids=list(range(N_CORES)))
    out = np.concatenate([res.results[c]["out"] for c in range(N_CORES)], axis=1)
    return out.reshape(B, T, N_TOTAL)


# revision 6
# speedup vs baseline: 1.1204x; 1.1204x over previous
"""Background-noise layer kernel for 8 Trainium2 NeuronCores.

Math (matches the reference): Poisson background spikes S (600, 10) with a
fixed RNG key, COO edge lists scattered into a dense weight matrix
W (250000, 10) (duplicates sum), output = S @ W^T reshaped to (1, 600, 250000).

Sharding: the neuron (output-feature) axis is split into 8 contiguous shards
of 31250. Each core holds its W-shard transposed (10, 31250) plus the tiny
replicated spike matrix transposed (10, 600), computes its (600, 31250) output
slice with TensorE matmuls (K=10 contraction on the partition axis), and
streams it straight out to DRAM. The kernel is output-write bound (~75 MB per
core), which is the memory roofline for this problem.

Host-side preprocessing is limited to RNG (the spikes are a constant — the
reference uses a fixed key 42, only implemented for threefry) and the COO →
dense scatter of the 1M edge weights (index bookkeeping; the dense W is what
the device GEMM consumes).
"""

import base64
import zlib

import numpy as np

B, T, U = 1, 600, 10
N_V1, N_LM = 200_000, 50_000
N_TOTAL = N_V1 + N_LM  # 250_000
N_CORES = 8
N_SHARD = N_TOTAL // N_CORES  # 31_250

# jax.random.poisson(jax.random.key(42, impl='threefry2x32'), 1.0, (600, 10))
# computed once offline; values are tiny ints (0..6). zlib+b64 of uint8 bytes.
_SPIKES_B64 = (
    "eJxNWAuy3DAIA+Ht/Y9co4/z+ma6u0nsYBCS7O7iv7mfP3TX/WwUevbH/X7/w73Ys3fQ9+peOntxr4N/"
    "++X02U89f5+rO8P9cp+ae2nHFSfcccUf99rh6zxrczq+hrPsYzeOnY0RQKGWn21Nowj2RQ1HN7PvvteG"
    "kdzQZtoD7wBsPHf4tEK/d/e1U6VkcGgf1NnRuxC9kolQNjbGgkPmrfndj8MrpfA2XmX0xgUmQdFjOJZL"
    "3JD3KWy6FFR3csTZD39gn9zYdyYtnhWpoyi4/vrm489WDqe1Ik4PZX4X384Il3yYCzg9SqvD3Tzea8P3"
    "Q0mCSsk5lJfO85mTd0eZI3yAMwpNEymFApLWqyQkOdgK8D4zAU89AuIXfDlaVWsD5Q0MRpPPvfGbnyJ3"
    "imdjcvjEwaD0HuOiIPwMZ6oKLHcUp95h+8/5dgbaw9QZ7ZaBcrQv/REeW0gmhi1wFIZhNa9T7gD4MScb"
    "Wr0wRNynHZiucjl0A6m9WnzfMgJ8YK5O3QinPcQV0/LrdSUbYdgreVK3TBLl3u0Uj9W6fQdM2kSdVA9M"
    "7Ag1N+rr+05fFHlI9WJU0/x1yi/aJzn7jGnLebpZVXCtDmyvlGSz0Dtqg1H2tor/RoXakkCrxHFXtRlS"
    "IbRTPHt3xF9KQqrh7GoR3UYuzA5oNwq4kBkTogu9te1HdOa4ffMpZ1HFfuQDM0XWzqgqqDH6hLDjJwhV"
    "hn5nI5ZZ2334iLuF2GHjhXIhGuoJuZIz+bYxXLc4wkSQT1TeBB8x+2jgj7MpvKcaYnoXLtzS5vJ2nkPC"
    "hJcGo93aqEd1kyf7VUc0KkL8qRZs53vxt9QEp9W1XOwKIOz1Pr+etJY4hksQB8AEgCdwgDvSjLeXz6ys"
    "aBliB0i+RtkYaYXR+spffokInpSjJFlh+3dcU4dFzbwg3t4DwhBILUdJkdRsyMffpgS9KwcE9uPx5WUx"
    "BURyXPN0Om5BE/oLEXwamRYqt5jbTDbjEIYprmFj8yHVMMaNjnq05CRE5yC3MJGegMsGQbSieRFS3PRP"
    "G/kdnBHHgtgIRSr79quW/VrLKDWFfWSsBSHsYjnH4I2rKLGCP7BqZT6pX+Z8+ooUzZJhUETL+o+IvWaQ"
    "GbMstrra7UkxK4qSSM+6r5gtxSQJ88n8WbOZztTw8W8ULkkW25C5xrzdf5ZisbRkuPyNBx8xmkON/wvD"
    "2nLQ2NCddNmSQgy14j3qYAuKvyW+XIL/qEtxkly6chyOadsrRwGRoCqM1KpNgmKdG8m/uFHZsDCNWMIv"
    "OCRU6oo8IVf93FVoroKrHpOI4DAauV6Oci4iwktYxZOm+SwwFfoWub9SWKS8cqiQWru91msZBc5MKAsi"
    "W/PKvliKXDYKUB8/ApL3rXEbxRhWf33UsqFPUVfEB/W3v5vlokSLw1zWeRwc48gBRMBK3dhA5QOPPZjm"
    "EdhHYr/v+bkvaEiqIl6ukIG07DhPe6CaZ4FsS/veL+GdWds23AkYfRfORm7Eb4vbmPKObbT/EqpQHw06"
    "DQcxpC4pm7JkO+wjVJXztZtNrfNJqTfdUDUgmxBgaF5vqVgOQFCY6IuZarrfdnDSd3LOvRKERzG89St"
    "ZjQhThwTLRSo9eKf6ab/obevk0XREsAqlpj+FS/c+U0E2JcG23CbsrrVtdEOct4v1bgWup7NqiuFeAnF"
    "sLNyE4b0uSqv3QPF+yrx2R8btyQ6DqmdggZW3sMREpzvwZOeUG7bg3mdFRCDDdhUzCSQTrTTWtHvekaM"
    "cupO5/5VKzDyCaff6wGG6bUjvP5uoO9OJdIwpyz7HtRNAype0dm7MjvWmrGumDdQTVKIsEAi1IxyGGNY"
    "y6WpwrEoH+uU9H0fMUY7lm6U4v5CGAM1V/HYdv/pbiKhwoIN4JZ80RMK2AbZfd2JDdepUiluVwxNmAPZ"
    "9E741yxAfM1FMfPCtnIF0LP0TrFuWEtnS3Ec/8I8zt3wfOyDRqqOe77bgx7SRBv7YEfDcIs3CUCd1AKl"
    "jEMj/6cnoAA1PI0cHnNG0bpfK13+eXIcJ2nDafpAhtYUzvhTIyS9vwyVmbs/Nr0hR8+vlzs9Tf0V8xBT"
    "4iv3tu2WA/HI3O0wHsUEBA56sbEinPygt5x0V7Bm1ehi398Lj2x4fw+lafU7AFNje082TXB6cMNUqHrX"
    "yeZDxDtpCNJ6PnagtSHv35e10xV25ExXfrK/e957VVdsdP/ng47OOcmVNSsLeOPdTNqCSwHhCm4t7/zz"
    "E5dhOz3JhUepiEBj4YM9d1abuboTbpQfauHgTE4yr1oOv9DFxIxueTpf2rgWvemdSWaoPOirWQNLxa+9"
    "jrJ1htM4BUuyjTg366Yrts5vEbjmSbhXtWBixaceDucqsOl3mCcNZNG3/6iBm7WVCh2netCJnU8oEbUL"
    "rmOH3eL0R4TUafG7Y3irK1MUQ5XBZ4x62be7+mKQ/53QbxdHHOH3a4+CjaUnRxNXMqWdWYoHiZnJsyCK"
    "JFXd1I6z001n8B+MpF8o="
)


def _spikes_t() -> np.ndarray:
    """Transposed spike matrix (U, T) float32."""
    raw = zlib.decompress(base64.b64decode(_SPIKES_B64))
    s = np.frombuffer(raw, dtype=np.uint8).astype(np.float32).reshape(T, U)
    return np.ascontiguousarray(s.T)


def _split_multi_waits(nc):
    """This environment's walrus rejects instructions carrying more than one
    sync-wait command ("Too many sync wait commands" in setupSyncWait). Tile
    freely attaches several waits to one instruction (e.g. a matmul waiting on
    two DMA-queue sems, or the kernel-tail drain waiting on every DMA lane).
    Post-pass: for every instruction with >1 wait, keep the first and move the
    rest onto fresh wait-only EventSemaphore instructions inserted immediately
    before it on the same engine. Waits are pre-execution conditions, so
    hoisting them onto same-engine predecessors inserted at that exact point
    preserves semantics."""
    import bass_rust

    ctr = 0
    for f in nc.m.functions:
        for bb in f.blocks:
            insts = bb.instructions  # live list
            new_list = None
            for ins in insts:
                si = getattr(ins, "sync_info", None)
                waits = list(si.on_wait) if si is not None else []
                if len(waits) > 1:
                    if new_list is None:
                        # copy of everything before this instruction
                        pos = insts.index(ins)
                        new_list = list(insts[:pos])
                    si.on_wait = [waits[0]]
                    for w in waits[1:]:
                        ctr += 1
                        ev = bass_rust.InstEventSemaphore(
                            name=f"wsplit_{ctr}",
                            engine=ins.engine,
                            ins=[],
                            outs=[],
                            sync_info=bass_rust.SyncInfo(on_wait=[w], on_update=[]),
                        )
                        new_list.append(ev)
                    new_list.append(ins)
                elif new_list is not None:
                    new_list.append(ins)
            if new_list is not None:
                insts[:] = new_list
    return ctr


_NC_CACHE = None


# Number of bf16 terms W is split into (W = sum of bf16 parts, spikes are
# small ints so exactly representable in bf16; products are exact, PSUM
# accumulates in fp32). 2 terms ≈ 1e-6 rel err; 3 terms ≈ fp32-exact.
# The fp32 PE path on this silicon runs ~8x slower (multi-pass), so bf16
# terms stacked along K is a large win; K = U * TERMS.
TERMS = 2
STRIP = 8192  # output staging strip width (fp32): 32KB/partition per buffer


def build_nc():
    """Per-core Bass program: out(600, 31250) = spk_stack.T @ w_stack.

    spk_stack (U*TERMS, 600) bf16, w_stack (U*TERMS, 31250) bf16 both stay
    resident in SBUF. TensorE produces (m-tile, 512) fp32 chunks in PSUM,
    DVE/ACT copy them into a (128, STRIP) SBUF strip, and one HWDGE DMA per
    (m-tile, strip) writes 4MB contiguous-rows blocks to DRAM. The kernel is
    output-DMA bound at ~330 GB/s/core (~92% of the per-NC HBM limit)."""
    global _NC_CACHE
    if _NC_CACHE is not None:
        return _NC_CACHE

    import concourse.bass as bass
    import concourse.mybir as mybir
    from concourse.tile import TileContext

    f32 = mybir.dt.float32
    bf16 = mybir.dt.bfloat16
    K = U * TERMS
    nc = bass.Bass(trn_type="TRN2")
    spk = nc.dram_tensor("spk", [K, T], bf16, kind="ExternalInput")
    wt = nc.dram_tensor("wt", [K, N_SHARD], bf16, kind="ExternalInput")
    out = nc.dram_tensor("out", [T, N_SHARD], f32, kind="ExternalOutput")

    m_tiles = [(m0, min(128, T - m0)) for m0 in range(0, T, 128)]
    strips = [(s0, min(STRIP, N_SHARD - s0)) for s0 in range(0, N_SHARD, STRIP)]

    with TileContext(nc) as tc:
        with (
            tc.tile_pool(name="const", bufs=1) as cpool,
            tc.tile_pool(name="stage", bufs=3) as stage,
            tc.tile_pool(name="psum", bufs=8, space="PSUM") as pp,
        ):
            spk_t = cpool.tile([K, T], bf16)
            nc.sync.dma_start(out=spk_t[:], in_=spk[:])
            w_t = cpool.tile([K, N_SHARD], bf16)
            nc.sync.dma_start(out=w_t[:], in_=wt[:])

            for m0, msz in m_tiles:
                for s0, ssz in strips:
                    ot = stage.tile([128, STRIP], f32)
                    for q0 in range(0, ssz, 512):
                        qsz = min(512, ssz - q0)
                        n0 = s0 + q0
                        ps = pp.tile([128, 512], f32)
                        nc.tensor.matmul(
                            ps[:msz, :qsz],
                            lhsT=spk_t[:, m0 : m0 + msz],
                            rhs=w_t[:, n0 : n0 + qsz],
                            start=True,
                            stop=True,
                        )
                        nc.any.tensor_copy(
                            out=ot[:msz, q0 : q0 + qsz], in_=ps[:msz, :qsz]
                        )
                    nc.sync.dma_start(
                        out=out[m0 : m0 + msz, s0 : s0 + ssz], in_=ot[:msz, :ssz]
                    )

    _split_multi_waits(nc)
    _NC_CACHE = nc
    return nc


def make_in_maps(w_v1, rows_v1, cols_v1, w_lm, rows_lm, cols_lm):
    """Host preprocessing: scatter COO edges into dense W, split into bf16
    terms, shard along neurons, transpose to (K, n) device layout."""
    import ml_dtypes

    w_v1 = np.asarray(w_v1, dtype=np.float32)
    w_lm = np.asarray(w_lm, dtype=np.float32)
    rows_v1 = np.asarray(rows_v1)
    cols_v1 = np.asarray(cols_v1)
    rows_lm = np.asarray(rows_lm)
    cols_lm = np.asarray(cols_lm)

    flat_v1 = rows_v1.astype(np.int64) * U + cols_v1.astype(np.int64)
    flat_lm = (rows_lm.astype(np.int64) + N_V1) * U + cols_lm.astype(np.int64)
    acc = np.bincount(flat_v1, weights=w_v1.astype(np.float64), minlength=N_TOTAL * U)
    acc += np.bincount(flat_lm, weights=w_lm.astype(np.float64), minlength=N_TOTAL * U)
    W = acc.astype(np.float32).reshape(N_TOTAL, U)

    # hi/lo bf16 split: W ≈ sum(parts); residual after TERMS terms ~2^(-9*TERMS)
    parts = []
    resid = W
    for _ in range(TERMS):
        p = resid.astype(ml_dtypes.bfloat16)
        parts.append(p)
        resid = resid - p.astype(np.float32)
    w_stack = np.concatenate(parts, axis=1)  # (N_TOTAL, U*TERMS) bf16

    spk_t = _spikes_t()  # (U, T) f32, small ints: exact in bf16
    spk_stack = np.tile(spk_t, (TERMS, 1)).astype(ml_dtypes.bfloat16)

    in_maps = []
    for c in range(N_CORES):
        w_shard_t = np.ascontiguousarray(w_stack[c * N_SHARD : (c + 1) * N_SHARD].T)
        in_maps.append({"spk": spk_stack, "wt": w_shard_t})
    return in_maps


def kernel(inp, w_v1, rows_v1, cols_v1, w_lm, rows_lm, cols_lm):
    from concourse.bass_utils import run_bass_kernel_spmd

    nc = build_nc()
    in_maps = make_in_maps(w_v1, rows_v1, cols_v1, w_lm, rows_lm, cols_lm)
    res = run_bass_kernel_spmd(nc, in_maps, core_ids=list(range(N_CORES)))
    out = np.concatenate([res.results[c]["out"] for c in range(N_CORES)], axis=1)
    return out.reshape(B, T, N_TOTAL)


# revision 7
# speedup vs baseline: 124.1725x; 110.8282x over previous
"""Background-noise layer kernel for 8 Trainium2 NeuronCores.

Math (matches the reference): Poisson background spikes S (600, 10) with a
fixed RNG key, COO edge lists scattered into a dense weight matrix
W (250000, 10) (duplicates sum), output = S @ W^T reshaped to (1, 600, 250000).

Sharding: the neuron (output-feature) axis is split into 8 contiguous shards
of 31250. Each core holds its W-shard transposed (10, 31250) plus the tiny
replicated spike matrix transposed (10, 600), computes its (600, 31250) output
slice with TensorE matmuls (K=10 contraction on the partition axis), and
streams it straight out to DRAM. The kernel is output-write bound (~75 MB per
core), which is the memory roofline for this problem.

Host-side preprocessing is limited to RNG (the spikes are a constant — the
reference uses a fixed key 42, only implemented for threefry) and the COO →
dense scatter of the 1M edge weights (index bookkeeping; the dense W is what
the device GEMM consumes).
"""

import base64
import zlib

import numpy as np

B, T, U = 1, 600, 10
N_V1, N_LM = 200_000, 50_000
N_TOTAL = N_V1 + N_LM  # 250_000
N_CORES = 8
N_SHARD = N_TOTAL // N_CORES  # 31_250

# jax.random.poisson(jax.random.key(42, impl='threefry2x32'), 1.0, (600, 10))
# computed once offline; values are tiny ints (0..6). zlib+b64 of uint8 bytes.
_SPIKES_B64 = (
    "eJxNWAuy3DAIA+Ht/Y9co4/z+ma6u0nsYBCS7O7iv7mfP3TX/WwUevbH/X7/w73Ys3fQ9+peOntxr4N/"
    "++X02U89f5+rO8P9cp+ae2nHFSfcccUf99rh6zxrczq+hrPsYzeOnY0RQKGWn21Nowj2RQ1HN7PvvteG"
    "kdzQZtoD7wBsPHf4tEK/d/e1U6VkcGgf1NnRuxC9kolQNjbGgkPmrfndj8MrpfA2XmX0xgUmQdFjOJZL"
    "3JD3KWy6FFR3csTZD39gn9zYdyYtnhWpoyi4/vrm489WDqe1Ik4PZX4X384Il3yYCzg9SqvD3Tzea8P3"
    "Q0mCSsk5lJfO85mTd0eZI3yAMwpNEymFApLWqyQkOdgK8D4zAU89AuIXfDlaVWsD5Q0MRpPPvfGbnyJ3"
    "imdjcvjEwaD0HuOiIPwMZ6oKLHcUp95h+8/5dgbaw9QZ7ZaBcrQv/REeW0gmhi1wFIZhNa9T7gD4MScb"
    "Wr0wRNynHZiucjl0A6m9WnzfMgJ8YK5O3QinPcQV0/LrdSUbYdgreVK3TBLl3u0Uj9W6fQdM2kSdVA9M"
    "7Ag1N+rr+05fFHlI9WJU0/x1yi/aJzn7jGnLebpZVXCtDmyvlGSz0Dtqg1H2tor/RoXakkCrxHFXtRlS"
    "IbRTPHt3xF9KQqrh7GoR3UYuzA5oNwq4kBkTogu9te1HdOa4ffMpZ1HFfuQDM0XWzqgqqDH6hLDjJwhV"
    "hn5nI5ZZ2334iLuF2GHjhXIhGuoJuZIz+bYxXLc4wkSQT1TeBB8x+2jgj7MpvKcaYnoXLtzS5vJ2nkPC"
    "hJcGo93aqEd1kyf7VUc0KkL8qRZs53vxt9QEp9W1XOwKIOz1Pr+etJY4hksQB8AEgCdwgDvSjLeXz6ys"
    "aBliB0i+RtkYaYXR+spffokInpSjJFlh+3dcU4dFzbwg3t4DwhBILUdJkdRsyMffpgS9KwcE9uPx5WUx"
    "BURyXPN0Om5BE/oLEXwamRYqt5jbTDbjEIYprmFj8yHVMMaNjnq05CRE5yC3MJGegMsGQbSieRFS3PRP"
    "G/kdnBHHgtgIRSr79quW/VrLKDWFfWSsBSHsYjnH4I2rKLGCP7BqZT6pX+Z8+ooUzZJhUETL+o+IvWaQ"
    "GbMstrra7UkxK4qSSM+6r5gtxSQJ88n8WbOZztTw8W8ULkkW25C5xrzdf5ZisbRkuPyNBx8xmkON/wvD"
    "2nLQ2NCddNmSQgy14j3qYAuKvyW+XIL/qEtxkly6chyOadsrRwGRoCqM1KpNgmKdG8m/uFHZsDCNWMIv"
    "OCRU6oo8IVf93FVoroKrHpOI4DAauV6Oci4iwktYxZOm+SwwFfoWub9SWKS8cqiQWru91msZBc5MKAsi"
    "W/PKvliKXDYKUB8/ApL3rXEbxRhWf33UsqFPUVfEB/W3v5vlokSLw1zWeRwc48gBRMBK3dhA5QOPPZjm"
    "EdhHYr/v+bkvaEiqIl6ukIG07DhPe6CaZ4FsS/veL+GdWds23AkYfRfORm7Eb4vbmPKObbT/EqpQHw06"
    "DQcxpC4pm7JkO+wjVJXztZtNrfNJqTfdUDUgmxBgaF5vqVgOQFCY6IuZarrfdnDSd3LOvRKERzG89St"
    "ZjQhThwTLRSo9eKf6ab/obevk0XREsAqlpj+FS/c+U0E2JcG23CbsrrVtdEOct4v1bgWup7NqiuFeAnF"
    "sLNyE4b0uSqv3QPF+yrx2R8btyQ6DqmdggZW3sMREpzvwZOeUG7bg3mdFRCDDdhUzCSQTrTTWtHvekaM"
    "cupO5/5VKzDyCaff6wGG6bUjvP5uoO9OJdIwpyz7HtRNAype0dm7MjvWmrGumDdQTVKIsEAi1IxyGGNY"
    "y6WpwrEoH+uU9H0fMUY7lm6U4v5CGAM1V/HYdv/pbiKhwoIN4JZ80RMK2AbZfd2JDdepUiluVwxNmAPZ"
    "9E741yxAfM1FMfPCtnIF0LP0TrFuWEtnS3Ec/8I8zt3wfOyDRqqOe77bgx7SRBv7YEfDcIs3CUCd1AKl"
    "jEMj/6cnoAA1PI0cHnNG0bpfK13+eXIcJ2nDafpAhtYUzvhTIyS9vwyVmbs/Nr0hR8+vlzs9Tf0V8xBT"
    "4iv3tu2WA/HI3O0wHsUEBA56sbEinPygt5x0V7Bm1ehi398Lj2x4fw+lafU7AFNje082TXB6cMNUqHrX"
    "yeZDxDtpCNJ6PnagtSHv35e10xV25ExXfrK/e957VVdsdP/ng47OOcmVNSsLeOPdTNqCSwHhCm4t7/zz"
    "E5dhOz3JhUepiEBj4YM9d1abuboTbpQfauHgTE4yr1oOv9DFxIxueTpf2rgWvemdSWaoPOirWQNLxa+9"
    "jrJ1htM4BUuyjTg366Yrts5vEbjmSbhXtWBixaceDucqsOl3mCcNZNG3/6iBm7WVCh2netCJnU8oEbUL"
    "rmOH3eL0R4TUafG7Y3irK1MUQ5XBZ4x62be7+mKQ/53QbxdHHOH3a4+CjaUnRxNXMqWdWYoHiZnJsyCK"
    "JFXd1I6z001n8B+MpF8o="
)


def _spikes_t() -> np.ndarray:
    """Transposed spike matrix (U, T) float32."""
    raw = zlib.decompress(base64.b64decode(_SPIKES_B64))
    s = np.frombuffer(raw, dtype=np.uint8).astype(np.float32).reshape(T, U)
    return np.ascontiguousarray(s.T)


def _split_multi_waits(nc):
    """This environment's walrus rejects instructions carrying more than one
    sync-wait command ("Too many sync wait commands" in setupSyncWait). Tile
    freely attaches several waits to one instruction (e.g. a matmul waiting on
    two DMA-queue sems, or the kernel-tail drain waiting on every DMA lane).
    Post-pass: for every instruction with >1 wait, keep the first and move the
    rest onto fresh wait-only EventSemaphore instructions inserted immediately
    before it on the same engine. Waits are pre-execution conditions, so
    hoisting them onto same-engine predecessors inserted at that exact point
    preserves semantics."""
    import bass_rust

    ctr = 0
    for f in nc.m.functions:
        for bb in f.blocks:
            insts = bb.instructions  # live list
            new_list = None
            for ins in insts:
                si = getattr(ins, "sync_info", None)
                waits = list(si.on_wait) if si is not None else []
                if len(waits) > 1:
                    if new_list is None:
                        # copy of everything before this instruction
                        pos = insts.index(ins)
                        new_list = list(insts[:pos])
                    si.on_wait = [waits[0]]
                    for w in waits[1:]:
                        ctr += 1
                        ev = bass_rust.InstEventSemaphore(
                            name=f"wsplit_{ctr}",
                            engine=ins.engine,
                            ins=[],
                            outs=[],
                            sync_info=bass_rust.SyncInfo(on_wait=[w], on_update=[]),
                        )
                        new_list.append(ev)
                    new_list.append(ins)
                elif new_list is not None:
                    new_list.append(ins)
            if new_list is not None:
                insts[:] = new_list
    return ctr


_NC_CACHE = None


# Number of bf16 terms W is split into (W = sum of bf16 parts, spikes are
# small ints so exactly representable in bf16; products are exact, PSUM
# accumulates in fp32). 2 terms ≈ 1e-6 rel err; 3 terms ≈ fp32-exact.
# The fp32 PE path on this silicon runs ~8x slower (multi-pass), so bf16
# terms stacked along K is a large win; K = U * TERMS.
TERMS = 3
STRIP = 8192  # output staging strip width (fp32): 32KB/partition per buffer


def build_nc():
    """Per-core Bass program: out(600, 31250) = spk_stack.T @ w_stack.

    spk_stack (U*TERMS, 600) bf16, w_stack (U*TERMS, 31250) bf16 both stay
    resident in SBUF. TensorE produces (m-tile, 512) fp32 chunks in PSUM,
    DVE/ACT copy them into a (128, STRIP) SBUF strip, and one HWDGE DMA per
    (m-tile, strip) writes 4MB contiguous-rows blocks to DRAM. The kernel is
    output-DMA bound at ~330 GB/s/core (~92% of the per-NC HBM limit)."""
    global _NC_CACHE
    if _NC_CACHE is not None:
        return _NC_CACHE

    import concourse.bass as bass
    import concourse.mybir as mybir
    from concourse.tile import TileContext

    f32 = mybir.dt.float32
    bf16 = mybir.dt.bfloat16
    K = U * TERMS
    nc = bass.Bass(trn_type="TRN2")
    spk = nc.dram_tensor("spk", [K, T], bf16, kind="ExternalInput")
    wt = nc.dram_tensor("wt", [K, N_SHARD], bf16, kind="ExternalInput")
    out = nc.dram_tensor("out", [T, N_SHARD], f32, kind="ExternalOutput")

    m_tiles = [(m0, min(128, T - m0)) for m0 in range(0, T, 128)]
    strips = [(s0, min(STRIP, N_SHARD - s0)) for s0 in range(0, N_SHARD, STRIP)]

    with TileContext(nc) as tc:
        with (
            tc.tile_pool(name="const", bufs=1) as cpool,
            tc.tile_pool(name="stage", bufs=4) as stage,
            tc.tile_pool(name="psum", bufs=8, space="PSUM") as pp,
        ):
            spk_t = cpool.tile([K, T], bf16)
            nc.sync.dma_start(out=spk_t[:], in_=spk[:])
            w_t = cpool.tile([K, N_SHARD], bf16)
            nc.sync.dma_start(out=w_t[:], in_=wt[:])

            for m0, msz in m_tiles:
                for s0, ssz in strips:
                    ot = stage.tile([128, STRIP], f32)
                    for q0 in range(0, ssz, 512):
                        qsz = min(512, ssz - q0)
                        n0 = s0 + q0
                        ps = pp.tile([128, 512], f32)
                        nc.tensor.matmul(
                            ps[:msz, :qsz],
                            lhsT=spk_t[:, m0 : m0 + msz],
                            rhs=w_t[:, n0 : n0 + qsz],
                            start=True,
                            stop=True,
                        )
                        nc.any.tensor_copy(
                            out=ot[:msz, q0 : q0 + qsz], in_=ps[:msz, :qsz]
                        )
                    nc.sync.dma_start(
                        out=out[m0 : m0 + msz, s0 : s0 + ssz], in_=ot[:msz, :ssz]
                    )

    _split_multi_waits(nc)
    _NC_CACHE = nc
    return nc


def make_in_maps(w_v1, rows_v1, cols_v1, w_lm, rows_lm, cols_lm):
    """Host preprocessing: scatter COO edges into dense W, split into bf16
    terms, shard along neurons, transpose to (K, n) device layout."""
    import ml_dtypes

    w_v1 = np.asarray(w_v1, dtype=np.float32)
    w_lm = np.asarray(w_lm, dtype=np.float32)
    rows_v1 = np.asarray(rows_v1)
    cols_v1 = np.asarray(cols_v1)
    rows_lm = np.asarray(rows_lm)
    cols_lm = np.asarray(cols_lm)

    flat_v1 = rows_v1.astype(np.int64) * U + cols_v1.astype(np.int64)
    flat_lm = (rows_lm.astype(np.int64) + N_V1) * U + cols_lm.astype(np.int64)
    acc = np.bincount(flat_v1, weights=w_v1.astype(np.float64), minlength=N_TOTAL * U)
    acc += np.bincount(flat_lm, weights=w_lm.astype(np.float64), minlength=N_TOTAL * U)
    W = acc.astype(np.float32).reshape(N_TOTAL, U)

    # hi/lo bf16 split: W ≈ sum(parts); residual after TERMS terms ~2^(-9*TERMS)
    parts = []
    resid = W
    for _ in range(TERMS):
        p = resid.astype(ml_dtypes.bfloat16)
        parts.append(p)
        resid = resid - p.astype(np.float32)
    w_stack = np.concatenate(parts, axis=1)  # (N_TOTAL, U*TERMS) bf16

    spk_t = _spikes_t()  # (U, T) f32, small ints: exact in bf16
    spk_stack = np.tile(spk_t, (TERMS, 1)).astype(ml_dtypes.bfloat16)

    in_maps = []
    for c in range(N_CORES):
        w_shard_t = np.ascontiguousarray(w_stack[c * N_SHARD : (c + 1) * N_SHARD].T)
        in_maps.append({"spk": spk_stack, "wt": w_shard_t})
    return in_maps


def kernel(inp, w_v1, rows_v1, cols_v1, w_lm, rows_lm, cols_lm):
    from concourse.bass_utils import run_bass_kernel_spmd

    nc = build_nc()
    in_maps = make_in_maps(w_v1, rows_v1, cols_v1, w_lm, rows_lm, cols_lm)
    res = run_bass_kernel_spmd(nc, in_maps, core_ids=list(range(N_CORES)))
    out = np.concatenate([res.results[c]["out"] for c in range(N_CORES)], axis=1)
    return out.reshape(B, T, N_TOTAL)
